# revision 1
# baseline (speedup 1.0000x reference)
"""Trainium2 Bass kernel for the Anisotropic Sliced-Wasserstein encoder
(segment_reduce): project [N,512] node features through [128,64] projections
(4 WL slices), sort each of the 256 projected columns within each of 1000
graph segments, and extract 100 quantiles per segment.

Strategy (8 NeuronCores, pure data-parallel over graphs, no collectives):
  host: stripe graphs across cores by segment-size rank (125 segments each);
        split the few largest segments across two sort slots (their sorted
        halves are merged exactly on the host), which bounds the padded slot
        length L at the k-th largest count; pad every slot to L with a
        synthetic node row that projects to +1e4 for every projection column
        (pads sort to the top and never collide with quantile ranks); pack
        columns element-major (col = elem*S + slot) and pre-transpose so the
        device sees xt [512, S*L] bf16 per core.
  dev:  DMA xt tiles -> PE matmul with the (scale-folded) projections ->
        evict PSUM (ScalarE, fp32->bf16) into two sort buffers
        [128 rows = (slice,proj), S*L] -> bitonic sort network
        (all-ascending reversal formulation, pruned to L, ping-pong between
        buffers: exactly two full-width DVE tensor_tensor min/max ops per
        round, every op 2x-mode eligible because the slot dim is innermost
        and contiguous) -> DMA the sorted buffers out.
  host: gather quantiles (ranks are host-known from `batch`) and assemble
        the [1000, 25600] float32 output.

Measured on silicon: ~1.20 ms whole-NEFF exec (DVE 95% busy at the
2 elem/cycle/lane tensor_tensor ceiling), scale-relative error 0.41%
(bf16 value rounding; monotone, so sort order and rank selection are exact).
"""
import numpy as np
import ml_dtypes

BF = ml_dtypes.bfloat16
NCORES = 8
G = 1000
POW = 2.0
BIG = 1e4


# ---------------------------------------------------------------------------
# Bitonic network descriptors (validated against np.sort).
# ---------------------------------------------------------------------------
def gen_rounds(L, n=None):
    if n is None:
        n = 1
        while n < L:
            n *= 2
    assert L % 2 == 0 and L <= n
    rounds = []
    m = 1
    while m < n:
        ops = []
        bs = 2 * m
        nb_full = L // bs
        if nb_full:
            ops.append(("cmpx", 0, 2 * m - 1, bs, nb_full, m, -1))
        b0 = nb_full * bs
        if b0 < L:
            i0 = max(0, b0 + 2 * m - L)
            if i0 < m and b0 + m < L:
                run = m - i0
                ops.append(("cmpx", b0 + i0, b0 + 2 * m - 1 - i0, 0, 1, run, -1))
                if i0 > 0:
                    ops.append(("copy", b0, 0, 1, i0))
            else:
                ops.append(("copy", b0, 0, 1, L - b0))
        rounds.append(ops)
        d = m // 2
        while d >= 1:
            ops = []
            bs = 2 * d
            nb_full = L // bs
            if nb_full:
                ops.append(("cmpx", 0, d, bs, nb_full, d, +1))
            b0 = nb_full * bs
            if b0 < L:
                run_p = max(0, L - b0 - d)
                if run_p:
                    ops.append(("cmpx", b0, b0 + d, 0, 1, run_p, +1))
                cs = b0 + run_p
                ce = min(b0 + d, L)
                if ce > cs:
                    ops.append(("copy", cs, 0, 1, ce - cs))
            rounds.append(ops)
            d //= 2
        m *= 2
    return rounds


# ---------------------------------------------------------------------------
# Device kernel
# ---------------------------------------------------------------------------
_NC_CACHE = {}


def _eview(bass_mod, buf_ap, base, off, bs, nb, run, rstep, ns):
    """View at columns base + (off + b*bs + r*rstep)*ns + [0..ns)."""
    part = list(buf_ap.ap[0])
    dims = [part]
    if nb > 1:
        dims.append([bs * ns, nb])
    dims.append([rstep * ns, run])
    dims.append([1, ns])
    return bass_mod.AP(buf_ap.tensor, buf_ap.offset + base + off * ns, dims)


def build_nc(groups, interleave=True):
    key = (tuple(groups), interleave)
    if key in _NC_CACHE:
        return _NC_CACHE[key]
    import concourse.bass as bass
    import concourse.bacc as bacc
    import concourse.mybir as mybir
    from concourse.tile import TileContext

    NCOL = sum(n * L for n, L in groups)
    bf = mybir.dt.bfloat16

    nc = bacc.Bacc("TRN2", target_bir_lowering=False, debug=False,
                   num_devices=NCORES)
    xt = nc.declare_dram_parameter("xt", [512, NCOL], bf, isOutput=False)
    proj = nc.declare_dram_parameter("proj", [128, 64], bf, isOutput=False)
    out = nc.declare_dram_parameter("sorted", [256, NCOL], bf, isOutput=True)

    MM = 512          # matmul free chunk == one PSUM bank (fp32)
    EV = 2048         # eviction chunk (4 banks)
    CH = 3072 if NCOL <= 30000 else 2048
    STAGE_BUFS = 2

    with TileContext(nc) as tc:
        with (
            tc.tile_pool(name="const", bufs=1) as constp,
            tc.tile_pool(name="stage", bufs=STAGE_BUFS) as stagep,
            tc.tile_pool(name="psum", bufs=2, space="PSUM") as psump,
            tc.tile_pool(name="bufs", bufs=1) as bufp,
        ):
            projt = constp.tile([128, 64], bf)
            nc.sync.dma_start(projt[:], proj[:])

            groups_rounds = [gen_rounds(L) for _, L in groups]
            nrounds = len(groups_rounds[0])
            bases = []
            b0 = 0
            for ns, L in groups:
                bases.append(b0)
                b0 += ns * L
            sizes = [ns * L for ns, L in groups]
            ngr = len(groups)

            bufsA = [bufp.tile([128, sizes[g]], bf, name=f"bufA{g}",
                               tag=f"bufA{g}") for g in range(ngr)]
            bufsB = [bufp.tile([128, sizes[g]], bf, name=f"bufB{g}",
                               tag=f"bufB{g}") for g in range(ngr)]
            bufsZ = [bufp.tile([128, sizes[g]], bf, name=f"bufZ{g}",
                               tag=f"bufZ{g}") for g in range(ngr)]

            def fill(b, tgts, split_evict=False):
                # Both slices of the pair are staged per chunk and projected
                # into one [128, EV] PSUM tile (slice ih in partitions
                # ih*64..), so each eviction uses all 128 lanes.
                nev = 0
                for g in range(ngr):
                    gb, gsz = bases[g], sizes[g]
                    c0 = 0
                    while c0 < gsz:
                        cw = min(CH, gsz - c0)
                        sts = []
                        for ih in (0, 1):
                            i = 2 * b + ih
                            st = stagep.tile([128, CH], bf, name=f"st{ih}",
                                             tag=f"st{ih}")
                            nc.sync.dma_start(
                                st[:, :cw],
                                xt[i * 128:(i + 1) * 128, gb + c0:gb + c0 + cw])
                            sts.append(st)
                        e0 = 0
                        while e0 < cw:
                            ew = min(EV, cw - e0)
                            ps = psump.tile([128, EV], mybir.dt.float32,
                                            name="ps", tag="ps")
                            for ih in (0, 1):
                                j0 = 0
                                while j0 < ew:
                                    jw = min(MM, ew - j0)
                                    nc.tensor.matmul(
                                        ps[64 * ih:64 * ih + 64, j0:j0 + jw],
                                        lhsT=projt[:],
                                        rhs=sts[ih][:, e0 + j0:e0 + j0 + jw],
                                        start=True, stop=True)
                                    j0 += jw
                            dst = tgts[g][:, c0 + e0:c0 + e0 + ew]
                            # For the first buffer the DVE is idle during
                            # fill: alternate evictions ACT/DVE.
                            if split_evict and nev % 2 == 1:
                                nc.vector.tensor_copy(dst, ps[:, :ew])
                            else:
                                nc.scalar.copy(dst, ps[:, :ew])
                            nev += 1
                            e0 += ew
                        c0 += cw

            def emit_round(A, Z, flip, ns, ops):
                cur, pong = (A, Z) if not flip else (Z, A)
                ca, pa = cur[:], pong[:]
                for op in ops:
                    if op[0] == "cmpx":
                        _, lo, hi, bs, nb, run, hstep = op
                        slo = _eview(bass, ca, 0, lo, bs, nb, run, +1, ns)
                        shi = _eview(bass, ca, 0, hi, bs, nb, run, hstep, ns)
                        dlo = _eview(bass, pa, 0, lo, bs, nb, run, +1, ns)
                        dhi = _eview(bass, pa, 0, hi, bs, nb, run, hstep, ns)
                        nc.vector.tensor_tensor(dlo, slo, shi,
                                                op=mybir.AluOpType.min)
                        nc.vector.tensor_tensor(dhi, slo, shi,
                                                op=mybir.AluOpType.max)
                    else:
                        _, off, bs, nb, run = op
                        src = _eview(bass, ca, 0, off, bs, nb, run, +1, ns)
                        dst = _eview(bass, pa, 0, off, bs, nb, run, +1, ns)
                        nc.vector.tensor_copy(dst, src)

            def _chunk_round(ops, e0, e1):
                """Restrict a round of uniform blocks (block stride bs from
                elem 0) to elems [e0, e1); e0/e1 must be multiples of every
                descriptor's bs. Copies and partial descriptors (nb==1 at the
                tail) go to the chunk containing them."""
                res = []
                for op in ops:
                    if op[0] == "copy":
                        if e0 <= op[1] < e1:
                            res.append(op)
                        continue
                    _, lo, hi, bs, nb, run, hstep = op
                    if nb == 1:
                        if e0 <= lo < e1:
                            res.append(op)
                        continue
                    assert e0 % bs == 0 and (e1 % bs == 0 or e1 >= bs * nb)
                    b0 = min(nb, (e0 + bs - 1) // bs)
                    b1 = min(nb, e1 // bs)
                    if b1 > b0:
                        res.append(("cmpx", lo + bs * b0, hi + bs * b0, bs,
                                    b1 - b0, run, hstep))
                return res

            def emit_sort(bufs, bufsZ_, split_first=False, tail_dma=None):
                L0 = groups[0][1]
                ns0 = groups[0][0]
                simple = (ngr == 1
                          and all(o[0] == "cmpx" and o[3] == 2 and o[5] == 1
                                  for o in groups_rounds[0][-1]))
                for r in range(nrounds):
                    last = r == nrounds - 1
                    for g in range(ngr):
                        ops = groups_rounds[g][r]
                        if g == 0 and ngr == 1 and split_first and r <= 9:
                            bsr = max(o[3] for o in ops if o[0] == "cmpx")
                            h = (L0 // 2) // bsr * bsr
                            emit_round(bufs[g], bufsZ_[g], r % 2, ns0,
                                       _chunk_round(ops, 0, h))
                            emit_round(bufs[g], bufsZ_[g], r % 2, ns0,
                                       _chunk_round(ops, h, L0))
                        elif (g == 0 and simple and tail_dma is not None
                              and last):
                            out_ap, row0 = tail_dma
                            nchunk = 3
                            step = (L0 // nchunk) // 2 * 2
                            cuts = [0] + [step * (k + 1) for k in range(nchunk - 1)] + [L0]
                            for k in range(nchunk):
                                e0, e1 = cuts[k], cuts[k + 1]
                                emit_round(bufs[g], bufsZ_[g], r % 2, ns0,
                                           _chunk_round(ops, e0, e1))
                                nc.sync.dma_start(
                                    out_ap[row0:row0 + 128,
                                           e0 * ns0:e1 * ns0],
                                    bufs[g][:, e0 * ns0:e1 * ns0])
                        else:
                            emit_round(bufs[g], bufsZ_[g], r % 2, groups[g][0],
                                       ops)
                return simple

            fill(0, bufsA, split_evict=False)
            fill(1, bufsB)
            emit_sort(bufsA, bufsZ, split_first=True)
            for g in range(ngr):
                nc.sync.dma_start(out[0:128, bases[g]:bases[g] + sizes[g]],
                                  bufsA[g][:])
            did_tail = emit_sort(bufsB, bufsZ, tail_dma=(out, 128))
            if not did_tail:
                for g in range(ngr):
                    nc.sync.dma_start(out[128:256, bases[g]:bases[g] + sizes[g]],
                                      bufsB[g][:])

    nc.finalize()
    _NC_CACHE[key] = nc
    return nc


# ---------------------------------------------------------------------------
# Host side
# ---------------------------------------------------------------------------
def _plan_split(counts, spc):
    """Choose slots-per-core S (even) and slot length L: the largest segments
    are split across two slots (host merges their sorted halves), bounding L
    below the global max count. Minimizes S*L."""
    cs = np.sort(counts)[::-1]
    best = None
    for extra in range(0, 4):                 # splits per core
        k = extra * NCORES                    # split segs (largest k)
        S = spc + extra
        S += S % 2
        Lmin = int(np.ceil((cs[0] + 1) / 2)) if k else 0
        L = max(int(cs[k]) if k < len(cs) else 2, Lmin, 2)
        L += L % 2
        if L * 2 < cs[0] + 1 and k == 0:
            continue
        cost = S * L
        if best is None or cost < best[0]:
            best = (cost, S, L, k)
    _, S, L, k = best
    return S, L, k


def _host_prepare(x, batch, projections, cum_weights, groups_override=None):
    N, DT = x.shape
    D, P = projections.shape
    I1 = DT // D
    Q = cum_weights.shape[0]
    counts = np.bincount(batch, minlength=G).astype(np.int64)
    starts = np.concatenate([[0], np.cumsum(counts)[:-1]]).astype(np.int64)
    spc = G // NCORES
    S, L, nsplit = _plan_split(counts, spc)
    if groups_override is not None:
        groups = list(groups_override)
        S = sum(n for n, _ in groups)
        L = max(Lg for _, Lg in groups)
        nsplit = 0
    else:
        groups = [(S, L)]

    qidx = np.floor(cum_weights[None, :].astype(np.float32)
                    * np.maximum(counts - 1, 0)[:, None].astype(np.float32)
                    ).astype(np.int64)
    scale = float((Q * P) ** (1.0 / POW))
    proj_s = np.ascontiguousarray(projections.astype(np.float32) / scale).astype(BF)
    proj_pad = np.zeros((128, 64), BF)
    proj_pad[:D, :P] = proj_s

    pf = projections.astype(np.float64)
    u_slice = pf @ np.linalg.solve(pf.T @ pf, np.full(P, BIG))
    u_row = np.tile(u_slice, I1).astype(np.float32)

    # stripe: global count-rank r -> core r % NCORES (ascending within core);
    # the nsplit largest segments land evenly on cores and are split in two.
    order = np.argsort(counts, kind="stable")
    split_set = set(order[G - nsplit:].tolist()) if nsplit else set()
    core_segs = [order[c::NCORES] for c in range(NCORES)]

    NCOL = sum(n * Lg for n, Lg in groups)
    in_maps = []
    slot_tables = []
    for c in range(NCORES):
        # slot table: (seg, start_within_seg, cnt_part); -1 seg = dummy pad
        slots = []
        for g in core_segs[c]:
            cg = int(counts[g])
            if g in split_set:
                c1 = (cg + 1) // 2
                slots.append((g, 0, c1))
                slots.append((g, c1, cg - c1))
            else:
                slots.append((g, 0, cg))
        while len(slots) < S:
            slots.append((-1, 0, 0))
        assert len(slots) == S, (len(slots), S)
        slot_tables.append(slots)
        seg_a = np.array([sl[0] for sl in slots])
        off_a = np.array([sl[1] for sl in slots])
        cnt_a = np.array([sl[2] for sl in slots])
        st_a = np.where(seg_a >= 0, starts[np.clip(seg_a, 0, None)] + off_a, 0)
        e = np.arange(L)[:, None]
        v = e < cnt_a[None, :]                         # [L, S]
        ix = np.where(v, st_a[None, :] + e, 0)
        cols = np.where(v.reshape(-1, 1), x[ix.reshape(-1)], u_row[None, :])
        xtc = np.ascontiguousarray(cols.T.astype(BF))  # [512, NCOL]
        in_maps.append({"xt": xtc, "proj": proj_pad})
    return in_maps, dict(groups=groups, S=S, L=L, spc=spc, qidx=qidx, Q=Q,
                         P=P, I1=I1, slot_tables=slot_tables, NCOL=NCOL,
                         counts=counts)


def _host_gather(sorted_list, meta):
    Q, P, I1, L, S = meta["Q"], meta["P"], meta["I1"], meta["L"], meta["S"]
    qidx = meta["qidx"]
    counts = meta["counts"]
    out = np.empty((G, I1 * Q * P), np.float32)
    for c, srt in enumerate(sorted_list):
        a = np.asarray(srt).astype(np.float32)         # [256, S*L]
        blk = a.reshape(2, 2, 64, L, S).transpose(0, 1, 2, 4, 3)  # [2,2,64,S,L]
        slots = meta["slot_tables"][c]
        # unsplit segments: direct rank gather
        one = [(si, sl[0]) for si, sl in enumerate(slots)
               if sl[0] >= 0 and sl[2] == counts[sl[0]]]
        if one:
            sidx = np.array([si for si, _ in one])
            segs = np.array([g for _, g in one])
            qs = qidx[segs]                            # [n, Q]
            sel = np.take_along_axis(blk[:, :, :, sidx, :],
                                     qs[None, None, None, :, :], axis=4)
            out[segs] = sel.transpose(3, 0, 1, 4, 2).reshape(len(segs),
                                                            I1 * Q * P)
        # split segments: merge the two sorted halves on host, then gather
        halves = {}
        for si, sl in enumerate(slots):
            if sl[0] >= 0 and sl[2] != counts[sl[0]]:
                halves.setdefault(sl[0], []).append((sl[1], si, sl[2]))
        for g, parts in halves.items():
            parts.sort()
            vals = np.concatenate([blk[:, :, :, si, :cnt]
                                   for _, si, cnt in parts], axis=3)
            vals = np.sort(vals, axis=3)               # [2,2,64,c_g]
            sel = vals[:, :, :, qidx[g]]               # [2,2,64,Q]
            out[g] = sel.transpose(0, 1, 3, 2).reshape(I1 * Q * P)
    return out


def _run_device(in_maps, groups, trace=False, tmpdir=None, interleave=True):
    from concourse.bass_utils import run_bass_kernel_spmd
    nc = build_nc(tuple(groups), interleave=interleave)
    res = run_bass_kernel_spmd(nc, in_maps, core_ids=list(range(NCORES)),
                               trace=trace, tmpdir=tmpdir)
    return res


def kernel(x, batch, projections, cum_weights):
    x = np.asarray(x, dtype=np.float32)
    batch = np.asarray(batch)
    projections = np.asarray(projections, dtype=np.float32)
    cum_weights = np.asarray(cum_weights, dtype=np.float32)
    in_maps, meta = _host_prepare(x, batch, projections, cum_weights)
    res = _run_device(in_maps, meta["groups"], trace=False)
    sorted_list = [res.results[c]["sorted"] for c in range(NCORES)]
    return _host_gather(sorted_list, meta)



# revision 5
# speedup vs baseline: 1.3771x; 1.3771x over previous
"""Trainium2 Bass kernel for the Anisotropic Sliced-Wasserstein encoder
(segment_reduce): project [N,512] node features through [128,64] projections
(4 WL slices), sort each of the 256 projected columns within each of 1000
graph segments, and extract 100 quantiles per segment.

Strategy (8 NeuronCores, data-parallel over graphs, no collectives):
  host: stripe graphs across cores by segment-size rank (S=128 slots each,
        largest segments split in two; sorted halves merged on host); slots
        ordered by DESCENDING count within each core so that pad cells
        (+BIG) form a lower-staircase in the slot dim; pack columns
        element-major (col = elem*S + slot); pre-transpose so the device
        sees xt [512, S*L] bf16 per core.
  dev:  PE matmul with scale-folded projections -> two sort buffers
        [128 rows, S*L] bf16 -> Batcher odd-even-merge sorting network
        (ascending comparators only; ping-pong buffers). Each network level
        is emitted as AP rectangles restricted by the count staircase:
        pad-pad cells are skipped, real-pad cells become ScalarE copies
        (min(real,BIG)=real), only real-real cells pay DVE tensor_tensor
        min/max. Invariant making this exact: with ascending comparators,
        positions >= cnt always hold +BIG and positions < cnt always hold
        real values. The restriction plan is computed from the across-core
        max envelope of slot counts (SPMD: one program for all cores).
  host: gather quantiles (ranks known from `batch`) and assemble the
        [1000, 25600] float32 output.
"""
import numpy as np
import ml_dtypes

BF = ml_dtypes.bfloat16
NCORES = 8
G = 1000
POW = 2.0
BIG = 1e4

DVE_CONST = 240.0
DVE_ROW = 0.04
DVE_EL = 0.6


# ---------------------------------------------------------------------------
# Batcher odd-even mergesort network, as AP-friendly descriptor streams
# ---------------------------------------------------------------------------
def oem_comparators(n):
    levels = []
    p = 1
    while p < n:
        k = p
        while k >= 1:
            cmps = []
            for j in range(k % p, n - k, 2 * k):
                for i in range(min(k, n - j - k)):
                    if (i + j) // (2 * p) == (i + j + k) // (2 * p):
                        cmps.append((i + j, i + j + k))
            levels.append(cmps)
            k //= 2
        p *= 2
    return levels


def gen_streams(L, n=256, e_flat=0):
    """Per level, a list of streams describing the comparator set.
      ('blk', x0, k, bs, nb, run): pairs (x0+b*bs+r, x0+b*bs+r+k)
      ('mrg', x0, k, bs2p, nsb, bs2k, nruns): merged-inner form (the slot
        dim is fused with the run dim -> no slot restriction possible).
    Superblocks fully below e_flat (where all slots are real anyway) use
    the merged form when per-sb emission would be too fragmented."""
    out = []
    p = 1
    while p < n:
        k = p
        while k >= 1:
            streams = []

            def add_runs(starts, k=k):
                full = [j for j in starts if j + 2 * k <= L]
                partial = [j for j in starts if j + k < L < j + 2 * k]
                while full:
                    stride = 2 * k
                    m = 1
                    while m < len(full) and full[m] == full[0] + m * stride:
                        m += 1
                    streams.append(('blk', full[0], k, stride, m, k))
                    full = full[m:]
                for j in partial:
                    streams.append(('blk', j, k, 1, 1, L - k - j))

            if k == p:
                add_runs(list(range(0, L - k, 2 * k)))
            else:
                nsb_total = (L + 2 * p - 1) // (2 * p)
                nruns = p // k - 1
                full_sb = 0
                while (full_sb + 1) * 2 * p <= L:
                    full_sb += 1
                mrg_sb = 0
                if nsb_total > 4:
                    lim = min(L, e_flat) if 2 * p >= 32 else L
                    while (mrg_sb + 1) * 2 * p <= lim:
                        mrg_sb += 1
                    if mrg_sb > 0:
                        streams.append(('mrg', k, k, 2 * p, mrg_sb, 2 * k, nruns))
                for sb in range(mrg_sb, full_sb):
                    add_runs([sb * 2 * p + k + 2 * k * u for u in range(nruns)])
                for sb in range(full_sb, nsb_total):
                    add_runs([sb * 2 * p + k + 2 * k * u for u in range(nruns)
                              if sb * 2 * p + k + 2 * k * u + k < L])
            out.append((p, k, streams))
            k //= 2
        p *= 2
    return out


def stream_pairs(st):
    if st[0] == 'blk':
        _, x0, k, bs, nb, run = st
        for b in range(nb):
            for r in range(run):
                yield (x0 + b * bs + r, x0 + b * bs + r + k)
    else:
        _, x0, k, bs2p, nsb, bs2k, nruns = st
        for sb in range(nsb):
            for u in range(nruns):
                for r in range(k):
                    yield (x0 + sb * bs2p + u * bs2k + r,
                           x0 + sb * bs2p + u * bs2k + r + k)


def validate_streams(L, n=256, e_flat=0):
    ref = oem_comparators(n)
    gen = gen_streams(L, n, e_flat=e_flat)
    for (refl, (p, k, sts)) in zip(ref, gen):
        want = sorted((a, b) for (a, b) in refl if b < L)
        got = sorted(pr for st in sts for pr in stream_pairs(st))
        assert got == want, ("oem stream gen mismatch", p, k)
    return gen


def build_plan(env_cnts, L, S, first_level_full=True, e_flat=None):
    """Item list per level. item = (kind, dims, lo_base, hi_base, K):
    kind 'tt' -> DVE min+max (both bases), 'cp' -> ScalarE copy lo->lo.
    dims = [(stride_cols, count), ...] outer->inner, <= 3 free dims."""
    env = np.sort(np.asarray(env_cnts))[::-1]
    assert len(env) == S

    def Keven(e):
        kk = int((env > e).sum())
        kk += kk % 2
        return min(S, kk)

    if e_flat is None:
        e_flat = int(env[env > 0].min()) if (env > 0).any() else 0
    levels = validate_streams(L, e_flat=e_flat)
    plan = []
    for li, (p, k, sts) in enumerate(levels):
        items = []
        touched = np.zeros(L, bool)
        for st in sts:
            for (a, b) in stream_pairs(st):
                touched[a] = touched[b] = True
            if st[0] == 'mrg':
                _, x0, kk, bs2p, nsb, bs2k, nruns = st
                dims = [(bs2p * S, nsb), (bs2k * S, nruns), (1, kk * S)]
                items.append(('tt', dims, x0 * S, (x0 + kk) * S, S))
                continue
            _, x0, kk, bs, nb, run = st
            if li == 0 and first_level_full:
                dims = [(bs * S, nb), (1, run * S)]
                items.append(('tt', dims, x0 * S, (x0 + kk) * S, S))
                continue
            if nb > 1:
                na, astride, pa = nb, bs, run
            else:
                na, astride, pa = run, 1, 1
            K1 = [Keven(x0 + a * astride + kk) for a in range(na)]
            K2 = [Keven(x0 + a * astride) for a in range(na)]
            INF = float('inf')
            best = [INF] * (na + 1)
            best[0] = 0.0
            choice = [None] * (na + 1)
            for a1 in range(1, na + 1):
                for a0 in range(a1 - 1, -1, -1):
                    K = K1[a0]
                    pairs = (a1 - a0) * pa
                    c = 0.0 if K == 0 else 2 * (DVE_CONST + DVE_ROW * pairs
                                                + DVE_EL * pairs * K)
                    if best[a0] + c < best[a1]:
                        best[a1] = best[a0] + c
                        choice[a1] = a0
            a1 = na
            rects = []
            while a1 > 0:
                a0 = choice[a1]
                rects.append((a0, a1))
                a1 = a0
            for (a0, a1) in reversed(rects):
                K = K1[a0]
                base = x0 + a0 * astride

                def mk(Kcols, koff):
                    dd = []
                    if a1 - a0 > 1:
                        dd.append((astride * S, a1 - a0))
                    if pa > 1:
                        dd.append((S, pa))
                    dd.append((1, Kcols))
                    return dd, (base + koff) * S
                if K > 0:
                    dims, b0c = mk(K, 0)
                    _, h0c = mk(K, kk)
                    items.append(('tt', dims, b0c, h0c, K))
                w2 = K2[a0]
                if w2 > K:
                    dims, b0c = mk(w2 - K, 0)
                    items.append(('cp', dims, b0c + K, None, w2 - K))
        # structural copies for positions untouched at this level
        un = np.nonzero(~touched)[0]
        segs = []
        for e in un:
            if segs and segs[-1][0] + segs[-1][1] == e:
                segs[-1][1] += 1
            else:
                segs.append([int(e), 1])
        fams = []
        for (st_, ln) in segs:
            if (fams and fams[-1][2] == ln
                    and fams[-1][3] != 0
                    and st_ - (fams[-1][0] + (fams[-1][1] - 1) * fams[-1][3])
                    == fams[-1][3]):
                fams[-1][1] += 1
            elif fams and fams[-1][1] == 1 and fams[-1][2] == ln:
                fams[-1][3] = st_ - fams[-1][0]
                fams[-1][1] = 2
            else:
                fams.append([int(st_), 1, int(ln), 0])
        for (f0, nf, ln, gap) in fams:
            K = S if (li == 0 and first_level_full) else Keven(f0)
            if K == 0:
                continue
            if nf == 1:
                dims = [(S, ln), (1, K)] if K < S else [(1, ln * S)]
            else:
                dims = ([(gap * S, nf), (S, ln), (1, K)] if K < S
                        else [(gap * S, nf), (1, ln * S)])
            items.append(('cp', dims, f0 * S, None, K))
        plan.append(items)
    return plan


def item_span(it):
    """(min_col, max_col) touched by an item, in column units."""
    kind, dims, lo, hi, K = it
    span = sum(st * (c - 1) for (st, c) in dims)
    if kind == 'tt':
        return (min(lo, hi), max(lo, hi) + span)
    return (lo, lo + span)


# ---------------------------------------------------------------------------
# Device kernel
# ---------------------------------------------------------------------------
_NC_CACHE = {}


def build_nc(env, L, S):
    key = (tuple(env), L, S)
    if key in _NC_CACHE:
        return _NC_CACHE[key]
    import concourse.bass as bass
    import concourse.bacc as bacc
    import concourse.mybir as mybir
    from concourse.tile import TileContext

    NCOL = S * L
    bf = mybir.dt.bfloat16
    plan = build_plan(np.asarray(env), L, S)

    nc = bacc.Bacc("TRN2", target_bir_lowering=False, debug=False,
                   num_devices=NCORES)
    xt = nc.declare_dram_parameter("xt", [512, NCOL], bf, isOutput=False)
    proj = nc.declare_dram_parameter("proj", [128, 64], bf, isOutput=False)
    out = nc.declare_dram_parameter("sorted", [256, NCOL], bf, isOutput=True)

    MM = 512          # matmul free chunk == one PSUM bank (fp32)
    EV = 2048         # eviction chunk (4 banks)
    CH = 3072 if NCOL <= 30000 else 2048

    with TileContext(nc) as tc:
        with (
            tc.tile_pool(name="const", bufs=1) as constp,
            tc.tile_pool(name="stage", bufs=2) as stagep,
            tc.tile_pool(name="psum", bufs=2, space="PSUM") as psump,
            tc.tile_pool(name="bufs", bufs=1) as bufp,
        ):
            projt = constp.tile([128, 64], bf)
            nc.sync.dma_start(projt[:], proj[:])

            bufA = bufp.tile([128, NCOL], bf, name="bufA", tag="bufA")
            bufB = bufp.tile([128, NCOL], bf, name="bufB", tag="bufB")
            bufZ = bufp.tile([128, NCOL], bf, name="bufZ", tag="bufZ")

            def fill(b, tgt, split_evict=False):
                """Generator: yields after each staged chunk so the caller
                can interleave emission with sort levels."""
                nev = 0
                c0 = 0
                while c0 < NCOL:
                    cw = min(CH, NCOL - c0)
                    sts = []
                    for ih in (0, 1):
                        i = 2 * b + ih
                        st = stagep.tile([128, CH], bf, name=f"st{ih}",
                                         tag=f"st{ih}")
                        nc.sync.dma_start(
                            st[:, :cw],
                            xt[i * 128:(i + 1) * 128, c0:c0 + cw])
                        sts.append(st)
                    e0 = 0
                    while e0 < cw:
                        ew = min(EV, cw - e0)
                        ps = psump.tile([128, EV], mybir.dt.float32,
                                        name="ps", tag="ps")
                        for ih in (0, 1):
                            j0 = 0
                            while j0 < ew:
                                jw = min(MM, ew - j0)
                                nc.tensor.matmul(
                                    ps[64 * ih:64 * ih + 64, j0:j0 + jw],
                                    lhsT=projt[:],
                                    rhs=sts[ih][:, e0 + j0:e0 + j0 + jw],
                                    start=True, stop=True)
                                j0 += jw
                        dst = tgt[:, c0 + e0:c0 + e0 + ew]
                        if split_evict and nev % 2 == 1:
                            nc.vector.tensor_copy(dst, ps[:, :ew])
                        else:
                            nc.scalar.copy(dst, ps[:, :ew])
                        nev += 1
                        e0 += ew
                    c0 += cw
                    yield

            def mkap(buf_ap, col, dims):
                part = list(buf_ap.ap[0])
                return bass.AP(buf_ap.tensor, buf_ap.offset + col,
                               [part] + [[st, c] for (st, c) in dims])

            def emit_item(it, ca, pa):
                kind, dims, lo, hi, K = it
                if kind == 'tt':
                    slo = mkap(ca, lo, dims)
                    shi = mkap(ca, hi, dims)
                    nc.vector.tensor_tensor(mkap(pa, lo, dims), slo, shi,
                                            op=mybir.AluOpType.min)
                    nc.vector.tensor_tensor(mkap(pa, hi, dims), slo, shi,
                                            op=mybir.AluOpType.max)
                else:
                    nc.scalar.copy(mkap(pa, lo, dims), mkap(ca, lo, dims))

            def emit_sort(buf, out_row0, fill_gen=None, fill_start=3):
                cur, pong = buf, bufZ
                nlv = len(plan)
                last_items = sorted(plan[-1], key=lambda it: item_span(it)[0])
                for li in range(nlv):
                    ca, pa = cur[:], pong[:]
                    if li == nlv - 1:
                        done_e = 0
                        n_it = len(last_items)
                        for ii, it in enumerate(last_items):
                            emit_item(it, ca, pa)
                            nxt = (item_span(last_items[ii + 1])[0] // S
                                   if ii + 1 < n_it else L)
                            if nxt - done_e >= 64 or (ii == n_it - 1 and
                                                      nxt > done_e):
                                nc.sync.dma_start(
                                    out[out_row0:out_row0 + 128,
                                        done_e * S:nxt * S],
                                    mkap(pa, done_e * S,
                                         [(1, (nxt - done_e) * S)]))
                                done_e = nxt
                    else:
                        for it in plan[li]:
                            emit_item(it, ca, pa)
                    if fill_gen is not None and li >= fill_start:
                        next(fill_gen, None)
                    cur, pong = pong, cur
                return cur

            for _ in fill(0, bufA, split_evict=True):
                pass
            emit_sort(bufA, 0, fill_gen=fill(1, bufB))
            emit_sort(bufB, 128)

    nc.finalize()
    _NC_CACHE[key] = nc
    return nc


# ---------------------------------------------------------------------------
# Host side
# ---------------------------------------------------------------------------
def _plan_split(counts, spc):
    """Choose slots-per-core S (even) and slot length L: the largest
    segments are split across two slots (host merges their sorted halves),
    bounding L below the global max count. Minimizes S*L."""
    cs = np.sort(counts)[::-1]
    best = None
    for extra in range(0, 4):
        k = extra * NCORES
        Sv = spc + extra
        Sv += Sv % 2
        Lmin = int(np.ceil((cs[0] + 1) / 2)) if k else 0
        Lv = max(int(cs[k]) if k < len(cs) else 2, Lmin, 2)
        Lv += Lv % 2
        if Lv * 2 < cs[0] + 1 and k == 0:
            continue
        cost = Sv * Lv
        if best is None or cost < best[0]:
            best = (cost, Sv, Lv, k)
    _, Sv, Lv, k = best
    return Sv, Lv, k


def _host_prepare(x, batch, projections, cum_weights):
    N, DT = x.shape
    D, P = projections.shape
    I1 = DT // D
    Q = cum_weights.shape[0]
    counts = np.bincount(batch, minlength=G).astype(np.int64)
    starts = np.concatenate([[0], np.cumsum(counts)[:-1]]).astype(np.int64)
    spc = G // NCORES
    S, L, nsplit = _plan_split(counts, spc)

    qidx = np.floor(cum_weights[None, :].astype(np.float32)
                    * np.maximum(counts - 1, 0)[:, None].astype(np.float32)
                    ).astype(np.int64)
    scale = float((Q * P) ** (1.0 / POW))
    proj_s = np.ascontiguousarray(
        projections.astype(np.float32) / scale).astype(BF)
    proj_pad = np.zeros((128, 64), BF)
    proj_pad[:D, :P] = proj_s

    pf = projections.astype(np.float64)
    u_slice = pf @ np.linalg.solve(pf.T @ pf, np.full(P, BIG))
    u_row = np.tile(u_slice, I1).astype(np.float32)

    order = np.argsort(counts, kind="stable")
    split_set = set(order[G - nsplit:].tolist()) if nsplit else set()
    core_segs = [order[c::NCORES] for c in range(NCORES)]

    NCOL = S * L
    in_maps = []
    slot_tables = []
    core_cnts = []
    for c in range(NCORES):
        slots = []
        for g in core_segs[c]:
            cg = int(counts[g])
            if g in split_set:
                c1 = (cg + 1) // 2
                slots.append((g, 0, c1))
                slots.append((g, c1, cg - c1))
            else:
                slots.append((g, 0, cg))
        slots.sort(key=lambda t: -t[2])   # descending count
        while len(slots) < S:
            slots.append((-1, 0, 0))
        assert len(slots) == S
        slot_tables.append(slots)
        cnt_a = np.array([sl[2] for sl in slots])
        core_cnts.append(cnt_a)
        seg_a = np.array([sl[0] for sl in slots])
        off_a = np.array([sl[1] for sl in slots])
        st_a = np.where(seg_a >= 0, starts[np.clip(seg_a, 0, None)] + off_a, 0)
        e = np.arange(L)[:, None]
        v = e < cnt_a[None, :]                         # [L, S]
        ix = np.where(v, st_a[None, :] + e, 0)
        cols = np.where(v.reshape(-1, 1), x[ix.reshape(-1)], u_row[None, :])
        xtc = np.ascontiguousarray(cols.T.astype(BF))  # [512, NCOL]
        in_maps.append({"xt": xtc, "proj": proj_pad})
    env = np.max(np.stack(core_cnts), axis=0)
    return in_maps, dict(env=env, S=S, L=L, qidx=qidx, Q=Q,
                         P=P, I1=I1, slot_tables=slot_tables, NCOL=NCOL,
                         counts=counts)


def _host_gather(sorted_list, meta):
    Q, P, I1, L, S = meta["Q"], meta["P"], meta["I1"], meta["L"], meta["S"]
    qidx = meta["qidx"]
    counts = meta["counts"]
    out = np.empty((G, I1 * Q * P), np.float32)
    for c, srt in enumerate(sorted_list):
        a = np.asarray(srt).astype(np.float32)         # [256, S*L]
        blk = a.reshape(2, 2, 64, L, S).transpose(0, 1, 2, 4, 3)
        slots = meta["slot_tables"][c]
        one = [(si, sl[0]) for si, sl in enumerate(slots)
               if sl[0] >= 0 and sl[2] == counts[sl[0]]]
        if one:
            sidx = np.array([si for si, _ in one])
            segs = np.array([g for _, g in one])
            qs = qidx[segs]                            # [n, Q]
            sel = np.take_along_axis(blk[:, :, :, sidx, :],
                                     qs[None, None, None, :, :], axis=4)
            out[segs] = sel.transpose(3, 0, 1, 4, 2).reshape(len(segs),
                                                             I1 * Q * P)
        halves = {}
        for si, sl in enumerate(slots):
            if sl[0] >= 0 and sl[2] != counts[sl[0]]:
                halves.setdefault(sl[0], []).append((sl[1], si, sl[2]))
        for g, parts in halves.items():
            parts.sort()
            vals = np.concatenate([blk[:, :, :, si, :cnt]
                                   for _, si, cnt in parts], axis=3)
            vals = np.sort(vals, axis=3)               # [2,2,64,c_g]
            sel = vals[:, :, :, qidx[g]]               # [2,2,64,Q]
            out[g] = sel.transpose(0, 1, 3, 2).reshape(I1 * Q * P)
    return out


def _run_device(in_maps, meta, trace=False, tmpdir=None):
    from concourse.bass_utils import run_bass_kernel_spmd
    nc = build_nc(meta["env"], meta["L"], meta["S"])
    res = run_bass_kernel_spmd(nc, in_maps, core_ids=list(range(NCORES)),
                               trace=trace, tmpdir=tmpdir)
    return res


def kernel(x, batch, projections, cum_weights):
    x = np.asarray(x, dtype=np.float32)
    batch = np.asarray(batch)
    projections = np.asarray(projections, dtype=np.float32)
    cum_weights = np.asarray(cum_weights, dtype=np.float32)
    in_maps, meta = _host_prepare(x, batch, projections, cum_weights)
    res = _run_device(in_maps, meta)
    sorted_list = [res.results[c]["sorted"] for c in range(NCORES)]
    return _host_gather(sorted_list, meta)


# revision 10
# speedup vs baseline: 1.4209x; 1.0318x over previous
"""Trainium2 Bass kernel for the Anisotropic Sliced-Wasserstein encoder
(segment_reduce): project [N,512] node features through [128,64] projections
(4 WL slices), sort each of the 256 projected columns within each of 1000
graph segments, and extract 100 quantiles per segment.

Strategy (8 NeuronCores, data-parallel over graphs, no collectives):
  host: stripe graphs across cores by segment-size rank (S=128 slots each,
        largest segments split in two; sorted halves merged on host); slots
        ordered by DESCENDING count within each core so that pad cells
        (+BIG) form a lower-staircase in the slot dim; pack columns
        element-major (col = elem*S + slot); pre-transpose so the device
        sees xt [512, S*L] bf16 per core.
  dev:  PE matmul with scale-folded projections -> two sort buffers
        [128 rows, S*L] bf16 -> Batcher odd-even-merge sorting network
        (ascending comparators only; ping-pong buffers). Each network level
        is emitted as AP rectangles restricted by the count staircase:
        pad-pad cells are skipped, real-pad cells become ScalarE copies
        (min(real,BIG)=real), only real-real cells pay DVE tensor_tensor
        min/max. Invariant making this exact: with ascending comparators,
        positions >= cnt always hold +BIG and positions < cnt always hold
        real values. The restriction plan is computed from the across-core
        max envelope of slot counts (SPMD: one program for all cores).
  host: gather quantiles (ranks known from `batch`) and assemble the
        [1000, 25600] float32 output.
"""
import numpy as np
import ml_dtypes

BF = ml_dtypes.bfloat16
NCORES = 8
G = 1000
POW = 2.0
BIG = 1e4

DVE_CONST = 150.0
DVE_ROW = 0.01
DVE_EL = 0.5


# ---------------------------------------------------------------------------
# Batcher odd-even mergesort network, as AP-friendly descriptor streams
# ---------------------------------------------------------------------------
def oem_comparators(n):
    levels = []
    p = 1
    while p < n:
        k = p
        while k >= 1:
            cmps = []
            for j in range(k % p, n - k, 2 * k):
                for i in range(min(k, n - j - k)):
                    if (i + j) // (2 * p) == (i + j + k) // (2 * p):
                        cmps.append((i + j, i + j + k))
            levels.append(cmps)
            k //= 2
        p *= 2
    return levels


def gen_streams(L, n=256, e_flat=0):
    """Per level, a list of streams describing the comparator set.
      ('blk', x0, k, bs, nb, run): pairs (x0+b*bs+r, x0+b*bs+r+k)
      ('mrg', x0, k, bs2p, nsb, bs2k, nruns): merged-inner form (the slot
        dim is fused with the run dim -> no slot restriction possible).
    Superblocks fully below e_flat (where all slots are real anyway) use
    the merged form when per-sb emission would be too fragmented."""
    out = []
    p = 1
    while p < n:
        k = p
        while k >= 1:
            streams = []

            def add_runs(starts, k=k):
                full = [j for j in starts if j + 2 * k <= L]
                partial = [j for j in starts if j + k < L < j + 2 * k]
                while full:
                    stride = 2 * k
                    m = 1
                    while m < len(full) and full[m] == full[0] + m * stride:
                        m += 1
                    streams.append(('blk', full[0], k, stride, m, k))
                    full = full[m:]
                for j in partial:
                    streams.append(('blk', j, k, 1, 1, L - k - j))

            if k == p:
                add_runs(list(range(0, L - k, 2 * k)))
            else:
                nsb_total = (L + 2 * p - 1) // (2 * p)
                nruns = p // k - 1
                full_sb = 0
                while (full_sb + 1) * 2 * p <= L:
                    full_sb += 1
                mrg_sb = 0
                if nsb_total > 4:
                    lim = min(L, e_flat) if 2 * p >= 32 else L
                    while (mrg_sb + 1) * 2 * p <= lim:
                        mrg_sb += 1
                    if mrg_sb > 0:
                        streams.append(('mrg', k, k, 2 * p, mrg_sb, 2 * k, nruns))
                for sb in range(mrg_sb, full_sb):
                    add_runs([sb * 2 * p + k + 2 * k * u for u in range(nruns)])
                for sb in range(full_sb, nsb_total):
                    add_runs([sb * 2 * p + k + 2 * k * u for u in range(nruns)
                              if sb * 2 * p + k + 2 * k * u + k < L])
            out.append((p, k, streams))
            k //= 2
        p *= 2
    return out


def stream_pairs(st):
    if st[0] == 'blk':
        _, x0, k, bs, nb, run = st
        for b in range(nb):
            for r in range(run):
                yield (x0 + b * bs + r, x0 + b * bs + r + k)
    else:
        _, x0, k, bs2p, nsb, bs2k, nruns = st
        for sb in range(nsb):
            for u in range(nruns):
                for r in range(k):
                    yield (x0 + sb * bs2p + u * bs2k + r,
                           x0 + sb * bs2p + u * bs2k + r + k)


def validate_streams(L, n=256, e_flat=0):
    ref = oem_comparators(n)
    gen = gen_streams(L, n, e_flat=e_flat)
    for (refl, (p, k, sts)) in zip(ref, gen):
        want = sorted((a, b) for (a, b) in refl if b < L)
        got = sorted(pr for st in sts for pr in stream_pairs(st))
        assert got == want, ("oem stream gen mismatch", p, k)
    return gen


def build_plan(env_cnts, L, S, first_level_full=True, e_flat=None):
    """Item list per level. item = (kind, dims, lo_base, hi_base, K):
    kind 'tt' -> DVE min+max (both bases), 'cp' -> ScalarE copy lo->lo.
    dims = [(stride_cols, count), ...] outer->inner, <= 3 free dims."""
    env = np.sort(np.asarray(env_cnts))[::-1]
    assert len(env) == S

    def Keven(e):
        kk = int((env > e).sum())
        kk += kk % 2
        return min(S, kk)

    if e_flat is None:
        e_flat = int(env[env > 0].min()) if (env > 0).any() else 0
    levels = validate_streams(L, e_flat=e_flat)
    plan = []
    for li, (p, k, sts) in enumerate(levels):
        items = []
        touched = np.zeros(L, bool)
        for st in sts:
            for (a, b) in stream_pairs(st):
                touched[a] = touched[b] = True
            if st[0] == 'mrg':
                _, x0, kk, bs2p, nsb, bs2k, nruns = st
                dims = [(bs2p * S, nsb), (bs2k * S, nruns), (1, kk * S)]
                items.append(('tt', dims, x0 * S, (x0 + kk) * S, S))
                continue
            _, x0, kk, bs, nb, run = st
            if li == 0 and first_level_full:
                dims = [(bs * S, nb), (1, run * S)]
                items.append(('tt', dims, x0 * S, (x0 + kk) * S, S))
                continue
            if nb > 1:
                na, astride, pa = nb, bs, run
            else:
                na, astride, pa = run, 1, 1
            K1 = [Keven(x0 + a * astride + kk) for a in range(na)]
            K2 = [Keven(x0 + a * astride) for a in range(na)]
            INF = float('inf')
            best = [INF] * (na + 1)
            best[0] = 0.0
            choice = [None] * (na + 1)
            for a1 in range(1, na + 1):
                for a0 in range(a1 - 1, -1, -1):
                    K = K1[a0]
                    pairs = (a1 - a0) * pa
                    c = 0.0 if K == 0 else 2 * (DVE_CONST + DVE_ROW * pairs
                                                + DVE_EL * pairs * K)
                    if best[a0] + c < best[a1]:
                        best[a1] = best[a0] + c
                        choice[a1] = a0
            a1 = na
            rects = []
            while a1 > 0:
                a0 = choice[a1]
                rects.append((a0, a1))
                a1 = a0
            for (a0, a1) in reversed(rects):
                K = K1[a0]
                base = x0 + a0 * astride

                def mk(Kcols, koff):
                    dd = []
                    if a1 - a0 > 1:
                        dd.append((astride * S, a1 - a0))
                    if pa > 1:
                        dd.append((S, pa))
                    dd.append((1, Kcols))
                    return dd, (base + koff) * S
                if K > 0:
                    dims, b0c = mk(K, 0)
                    _, h0c = mk(K, kk)
                    items.append(('tt', dims, b0c, h0c, K))
                w2 = K2[a0]
                if w2 > K:
                    dims, b0c = mk(w2 - K, 0)
                    items.append(('cp', dims, b0c + K, None, w2 - K))
        # structural copies for positions untouched at this level
        un = np.nonzero(~touched)[0]
        segs = []
        for e in un:
            if segs and segs[-1][0] + segs[-1][1] == e:
                segs[-1][1] += 1
            else:
                segs.append([int(e), 1])
        fams = []
        for (st_, ln) in segs:
            if (fams and fams[-1][2] == ln
                    and fams[-1][3] != 0
                    and st_ - (fams[-1][0] + (fams[-1][1] - 1) * fams[-1][3])
                    == fams[-1][3]):
                fams[-1][1] += 1
            elif fams and fams[-1][1] == 1 and fams[-1][2] == ln:
                fams[-1][3] = st_ - fams[-1][0]
                fams[-1][1] = 2
            else:
                fams.append([int(st_), 1, int(ln), 0])
        for (f0, nf, ln, gap) in fams:
            K = S if (li == 0 and first_level_full) else Keven(f0)
            if K == 0:
                continue
            if nf == 1:
                dims = [(S, ln), (1, K)] if K < S else [(1, ln * S)]
            else:
                dims = ([(gap * S, nf), (S, ln), (1, K)] if K < S
                        else [(gap * S, nf), (1, ln * S)])
            items.append(('cp', dims, f0 * S, None, K))
        # balance copy load: ScalarE runs ~1.25 cyc/elem @1.2GHz; when a
        # level's copy time would exceed ~70% of its DVE time, move the
        # largest copies to DVE as u32 tensor_copy (~0.31 cyc/elem @0.96).
        dve_ns = sum(2 * (DVE_CONST + DVE_EL * int(np.prod([c for _, c in d])))
                     for (kind, d, *_r) in [(i[0], i[1]) for i in items]
                     if kind == 'tt') / 0.96
        cps = [i for i in items if i[0] == 'cp']
        cps.sort(key=lambda i: -int(np.prod([c for _, c in i[1]])))
        act_ns = sum((260 + 1.25 * int(np.prod([c for _, c in i[1]]))) / 1.2
                     for i in cps)
        moved = set()
        for i in cps:
            if act_ns <= 0.7 * dve_ns:
                break
            fdv = int(np.prod([c for _, c in i[1]]))
            act_ns -= (260 + 1.25 * fdv) / 1.2
            moved.add(id(i))
        items = [(('cpd',) + i[1:]) if (i[0] == 'cp' and id(i) in moved)
                 else i for i in items]
        plan.append(items)
    return plan


def item_span(it):
    """(min_col, max_col) touched by an item, in column units."""
    kind, dims, lo, hi, K = it
    span = sum(st * (c - 1) for (st, c) in dims)
    if kind == 'tt':
        return (min(lo, hi), max(lo, hi) + span)
    return (lo, lo + span)


# ---------------------------------------------------------------------------
# Device kernel
# ---------------------------------------------------------------------------
_NC_CACHE = {}


def build_nc(env, L, S):
    key = (tuple(env), L, S)
    if key in _NC_CACHE:
        return _NC_CACHE[key]
    import concourse.bass as bass
    import concourse.bacc as bacc
    import concourse.mybir as mybir
    from concourse.tile import TileContext

    NCOL = S * L
    bf = mybir.dt.bfloat16
    plan = build_plan(np.asarray(env), L, S)

    nc = bacc.Bacc("TRN2", target_bir_lowering=False, debug=False,
                   num_devices=NCORES)
    xt = nc.declare_dram_parameter("xt", [512, NCOL], bf, isOutput=False)
    proj = nc.declare_dram_parameter("proj", [128, 64], bf, isOutput=False)
    out = nc.declare_dram_parameter("sorted", [256, NCOL], bf, isOutput=True)

    MM = 512          # matmul free chunk == one PSUM bank (fp32)
    EV = 2048         # eviction chunk (4 banks)
    CH = 3072 if NCOL <= 30000 else 2048

    with TileContext(nc) as tc:
        with (
            tc.tile_pool(name="const", bufs=1) as constp,
            tc.tile_pool(name="stage", bufs=2) as stagep,
            tc.tile_pool(name="psum", bufs=2, space="PSUM") as psump,
            tc.tile_pool(name="bufs", bufs=1) as bufp,
        ):
            projt = constp.tile([128, 64], bf)
            nc.sync.dma_start(projt[:], proj[:])

            bufA = bufp.tile([128, NCOL], bf, name="bufA", tag="bufA")
            bufB = bufp.tile([128, NCOL], bf, name="bufB", tag="bufB")
            bufZ = bufp.tile([128, NCOL], bf, name="bufZ", tag="bufZ")

            def fill(b, tgt, split_evict=False):
                """Generator: yields after each staged chunk so the caller
                can interleave emission with sort levels."""
                nev = 0
                c0 = 0
                while c0 < NCOL:
                    cw = min(CH, NCOL - c0)
                    sts = []
                    for ih in (0, 1):
                        i = 2 * b + ih
                        st = stagep.tile([128, CH], bf, name=f"st{ih}",
                                         tag=f"st{ih}")
                        nc.sync.dma_start(
                            st[:, :cw],
                            xt[i * 128:(i + 1) * 128, c0:c0 + cw])
                        sts.append(st)
                    e0 = 0
                    while e0 < cw:
                        ew = min(EV, cw - e0)
                        ps = psump.tile([128, EV], mybir.dt.float32,
                                        name="ps", tag="ps")
                        for ih in (0, 1):
                            j0 = 0
                            while j0 < ew:
                                jw = min(MM, ew - j0)
                                nc.tensor.matmul(
                                    ps[64 * ih:64 * ih + 64, j0:j0 + jw],
                                    lhsT=projt[:],
                                    rhs=sts[ih][:, e0 + j0:e0 + j0 + jw],
                                    start=True, stop=True)
                                j0 += jw
                        dst = tgt[:, c0 + e0:c0 + e0 + ew]
                        if split_evict and nev % 2 == 1:
                            nc.vector.tensor_copy(dst, ps[:, :ew])
                        else:
                            nc.scalar.copy(dst, ps[:, :ew])
                        nev += 1
                        e0 += ew
                    yield (c0, cw)
                    c0 += cw

            def mkap(buf_ap, col, dims):
                part = list(buf_ap.ap[0])
                return bass.AP(buf_ap.tensor, buf_ap.offset + col,
                               [part] + [[st, c] for (st, c) in dims])

            def emit_item(it, ca, pa):
                kind, dims, lo, hi, K = it
                if kind == 'tt':
                    slo = mkap(ca, lo, dims)
                    shi = mkap(ca, hi, dims)
                    nc.vector.tensor_tensor(mkap(pa, lo, dims), slo, shi,
                                            op=mybir.AluOpType.min)
                    nc.vector.tensor_tensor(mkap(pa, hi, dims), slo, shi,
                                            op=mybir.AluOpType.max)
                elif kind == 'cpd':
                    nc.vector.tensor_copy(
                        mkap(pa, lo, dims).bitcast(mybir.dt.uint32),
                        mkap(ca, lo, dims).bitcast(mybir.dt.uint32))
                else:
                    nc.scalar.copy(mkap(pa, lo, dims), mkap(ca, lo, dims))

            def emit_sort(cur, pong, out_row0, fill_gen=None, fill_start=3,
                          start_level=0):
                nlv = len(plan)
                last_items = sorted(plan[-1], key=lambda it: item_span(it)[0])
                for li in range(start_level, nlv):
                    ca, pa = cur[:], pong[:]
                    if li == nlv - 1:
                        done_e = 0
                        n_it = len(last_items)
                        for ii, it in enumerate(last_items):
                            emit_item(it, ca, pa)
                            nxt = (item_span(last_items[ii + 1])[0] // S
                                   if ii + 1 < n_it else L)
                            thr = 48 if done_e < L - 56 else 16
                            if nxt - done_e >= thr or (ii == n_it - 1 and
                                                       nxt > done_e):
                                nc.sync.dma_start(
                                    out[out_row0:out_row0 + 128,
                                        done_e * S:nxt * S],
                                    mkap(pa, done_e * S,
                                         [(1, (nxt - done_e) * S)]))
                                done_e = nxt
                    else:
                        for it in plan[li]:
                            emit_item(it, ca, pa)
                    if fill_gen is not None and li >= fill_start:
                        next(fill_gen, None)
                    cur, pong = pong, cur

            # fill A; interleave sort-A level 0 (pairs (2i,2i+1), full slot
            # width) chunk-by-chunk behind the PSUM evictions
            assert L % 2 == 0
            for (c0, cw) in fill(0, bufA, split_evict=True):
                ne = cw // S
                assert ne % 2 == 0 and cw % S == 0
                dims = [(2 * S, ne // 2), (1, S)]
                slo = mkap(bufA[:], c0, dims)
                shi = mkap(bufA[:], c0 + S, dims)
                nc.vector.tensor_tensor(mkap(bufZ[:], c0, dims), slo, shi,
                                        op=mybir.AluOpType.min)
                nc.vector.tensor_tensor(mkap(bufZ[:], c0 + S, dims), slo, shi,
                                        op=mybir.AluOpType.max)
            emit_sort(bufZ, bufA, 0, fill_gen=fill(1, bufB), start_level=1)
            emit_sort(bufB, bufZ, 128)

    nc.finalize()
    _NC_CACHE[key] = nc
    return nc


# ---------------------------------------------------------------------------
# Host side
# ---------------------------------------------------------------------------
def _plan_split(counts, spc):
    """Choose slots-per-core S (even) and slot length L: the largest
    segments are split across two slots (host merges their sorted halves),
    bounding L below the global max count. Minimizes S*L."""
    cs = np.sort(counts)[::-1]
    best = None
    for extra in range(0, 4):
        k = extra * NCORES
        Sv = spc + extra
        Sv += Sv % 2
        Lmin = int(np.ceil((cs[0] + 1) / 2)) if k else 0
        Lv = max(int(cs[k]) if k < len(cs) else 2, Lmin, 2)
        Lv += Lv % 2
        if Lv * 2 < cs[0] + 1 and k == 0:
            continue
        cost = Sv * Lv
        if best is None or cost < best[0]:
            best = (cost, Sv, Lv, k)
    _, Sv, Lv, k = best
    return Sv, Lv, k


def _host_prepare(x, batch, projections, cum_weights):
    N, DT = x.shape
    D, P = projections.shape
    I1 = DT // D
    Q = cum_weights.shape[0]
    counts = np.bincount(batch, minlength=G).astype(np.int64)
    starts = np.concatenate([[0], np.cumsum(counts)[:-1]]).astype(np.int64)
    spc = G // NCORES
    S, L, nsplit = _plan_split(counts, spc)

    qidx = np.floor(cum_weights[None, :].astype(np.float32)
                    * np.maximum(counts - 1, 0)[:, None].astype(np.float32)
                    ).astype(np.int64)
    scale = float((Q * P) ** (1.0 / POW))
    proj_s = np.ascontiguousarray(
        projections.astype(np.float32) / scale).astype(BF)
    proj_pad = np.zeros((128, 64), BF)
    proj_pad[:D, :P] = proj_s

    pf = projections.astype(np.float64)
    u_slice = pf @ np.linalg.solve(pf.T @ pf, np.full(P, BIG))
    u_row = np.tile(u_slice, I1).astype(np.float32)

    order = np.argsort(counts, kind="stable")
    split_set = set(order[G - nsplit:].tolist()) if nsplit else set()
    core_segs = [order[c::NCORES] for c in range(NCORES)]

    NCOL = S * L
    in_maps = []
    slot_tables = []
    core_cnts = []
    for c in range(NCORES):
        slots = []
        for g in core_segs[c]:
            cg = int(counts[g])
            if g in split_set:
                c1 = (cg + 1) // 2
                slots.append((g, 0, c1))
                slots.append((g, c1, cg - c1))
            else:
                slots.append((g, 0, cg))
        slots.sort(key=lambda t: -t[2])   # descending count
        while len(slots) < S:
            slots.append((-1, 0, 0))
        assert len(slots) == S
        slot_tables.append(slots)
        cnt_a = np.array([sl[2] for sl in slots])
        core_cnts.append(cnt_a)
        seg_a = np.array([sl[0] for sl in slots])
        off_a = np.array([sl[1] for sl in slots])
        st_a = np.where(seg_a >= 0, starts[np.clip(seg_a, 0, None)] + off_a, 0)
        e = np.arange(L)[:, None]
        v = e < cnt_a[None, :]                         # [L, S]
        ix = np.where(v, st_a[None, :] + e, 0)
        cols = np.where(v.reshape(-1, 1), x[ix.reshape(-1)], u_row[None, :])
        xtc = np.ascontiguousarray(cols.T.astype(BF))  # [512, NCOL]
        in_maps.append({"xt": xtc, "proj": proj_pad})
    env = np.max(np.stack(core_cnts), axis=0)
    return in_maps, dict(env=env, S=S, L=L, qidx=qidx, Q=Q,
                         P=P, I1=I1, slot_tables=slot_tables, NCOL=NCOL,
                         counts=counts)


def _host_gather(sorted_list, meta):
    Q, P, I1, L, S = meta["Q"], meta["P"], meta["I1"], meta["L"], meta["S"]
    qidx = meta["qidx"]
    counts = meta["counts"]
    out = np.empty((G, I1 * Q * P), np.float32)
    for c, srt in enumerate(sorted_list):
        a = np.asarray(srt).astype(np.float32)         # [256, S*L]
        blk = a.reshape(2, 2, 64, L, S).transpose(0, 1, 2, 4, 3)
        slots = meta["slot_tables"][c]
        one = [(si, sl[0]) for si, sl in enumerate(slots)
               if sl[0] >= 0 and sl[2] == counts[sl[0]]]
        if one:
            sidx = np.array([si for si, _ in one])
            segs = np.array([g for _, g in one])
            qs = qidx[segs]                            # [n, Q]
            sel = np.take_along_axis(blk[:, :, :, sidx, :],
                                     qs[None, None, None, :, :], axis=4)
            out[segs] = sel.transpose(3, 0, 1, 4, 2).reshape(len(segs),
                                                             I1 * Q * P)
        halves = {}
        for si, sl in enumerate(slots):
            if sl[0] >= 0 and sl[2] != counts[sl[0]]:
                halves.setdefault(sl[0], []).append((sl[1], si, sl[2]))
        for g, parts in halves.items():
            parts.sort()
            vals = np.concatenate([blk[:, :, :, si, :cnt]
                                   for _, si, cnt in parts], axis=3)
            vals = np.sort(vals, axis=3)               # [2,2,64,c_g]
            sel = vals[:, :, :, qidx[g]]               # [2,2,64,Q]
            out[g] = sel.transpose(0, 1, 3, 2).reshape(I1 * Q * P)
    return out


def _run_device(in_maps, meta, trace=False, tmpdir=None):
    from concourse.bass_utils import run_bass_kernel_spmd
    nc = build_nc(meta["env"], meta["L"], meta["S"])
    res = run_bass_kernel_spmd(nc, in_maps, core_ids=list(range(NCORES)),
                               trace=trace, tmpdir=tmpdir)
    return res


def kernel(x, batch, projections, cum_weights):
    x = np.asarray(x, dtype=np.float32)
    batch = np.asarray(batch)
    projections = np.asarray(projections, dtype=np.float32)
    cum_weights = np.asarray(cum_weights, dtype=np.float32)
    in_maps, meta = _host_prepare(x, batch, projections, cum_weights)
    res = _run_device(in_maps, meta)
    sorted_list = [res.results[c]["sorted"] for c in range(NCORES)]
    return _host_gather(sorted_list, meta)


# revision 18
# speedup vs baseline: 1.4572x; 1.0255x over previous
"""Trainium2 Bass kernel for the Anisotropic Sliced-Wasserstein encoder
(segment_reduce): project [N,512] node features through [128,64] projections
(4 WL slices), sort each of the 256 projected columns within each of 1000
graph segments, and extract 100 quantiles per segment.

Strategy (8 NeuronCores, data-parallel over graphs, no collectives):
  host: stripe graphs across cores by segment-size rank (S=128 slots each,
        largest segments split in two; sorted halves merged on host); slots
        ordered by DESCENDING count within each core so that pad cells
        (+BIG) form a lower-staircase in the slot dim; pack columns
        element-major (col = elem*S + slot); pre-transpose so the device
        sees xt [512, S*L] bf16 per core.
  dev:  PE matmul with scale-folded projections -> two sort buffers
        [128 rows, S*L] bf16 -> Batcher odd-even-merge sorting network
        (ascending comparators only; ping-pong buffers). Each network level
        is emitted as AP rectangles restricted by the count staircase:
        pad-pad cells are skipped, real-pad cells become ScalarE copies
        (min(real,BIG)=real), only real-real cells pay DVE tensor_tensor
        min/max. Invariant making this exact: with ascending comparators,
        positions >= cnt always hold +BIG and positions < cnt always hold
        real values. The restriction plan is computed from the across-core
        max envelope of slot counts (SPMD: one program for all cores).
  host: gather quantiles (ranks known from `batch`) and assemble the
        [1000, 25600] float32 output.
"""
import numpy as np
import ml_dtypes

BF = ml_dtypes.bfloat16
NCORES = 8
G = 1000
POW = 2.0
BIG = 1e4

DVE_CONST = 150.0
DVE_ROW = 0.01
DVE_EL = 0.5


# ---------------------------------------------------------------------------
# Batcher odd-even mergesort network, as AP-friendly descriptor streams
# ---------------------------------------------------------------------------
def oem_comparators(n):
    levels = []
    p = 1
    while p < n:
        k = p
        while k >= 1:
            cmps = []
            for j in range(k % p, n - k, 2 * k):
                for i in range(min(k, n - j - k)):
                    if (i + j) // (2 * p) == (i + j + k) // (2 * p):
                        cmps.append((i + j, i + j + k))
            levels.append(cmps)
            k //= 2
        p *= 2
    return levels


def gen_streams(L, n=256, e_flat=0):
    """Per level, a list of streams describing the comparator set.
      ('blk', x0, k, bs, nb, run): pairs (x0+b*bs+r, x0+b*bs+r+k)
      ('mrg', x0, k, bs2p, nsb, bs2k, nruns): merged-inner form (the slot
        dim is fused with the run dim -> no slot restriction possible).
    Superblocks fully below e_flat (where all slots are real anyway) use
    the merged form when per-sb emission would be too fragmented."""
    out = []
    p = 1
    while p < n:
        k = p
        while k >= 1:
            streams = []

            def add_runs(starts, k=k):
                full = [j for j in starts if j + 2 * k <= L]
                partial = [j for j in starts if j + k < L < j + 2 * k]
                while full:
                    stride = 2 * k
                    m = 1
                    while m < len(full) and full[m] == full[0] + m * stride:
                        m += 1
                    streams.append(('blk', full[0], k, stride, m, k))
                    full = full[m:]
                for j in partial:
                    streams.append(('blk', j, k, 1, 1, L - k - j))

            if k == p:
                add_runs(list(range(0, L - k, 2 * k)))
            else:
                nsb_total = (L + 2 * p - 1) // (2 * p)
                nruns = p // k - 1
                full_sb = 0
                while (full_sb + 1) * 2 * p <= L:
                    full_sb += 1
                mrg_sb = 0
                if nsb_total > 4:
                    lim = min(L, e_flat) if 2 * p >= 32 else L
                    while (mrg_sb + 1) * 2 * p <= lim:
                        mrg_sb += 1
                    if mrg_sb > 0:
                        streams.append(('mrg', k, k, 2 * p, mrg_sb, 2 * k, nruns))
                for sb in range(mrg_sb, full_sb):
                    add_runs([sb * 2 * p + k + 2 * k * u for u in range(nruns)])
                for sb in range(full_sb, nsb_total):
                    add_runs([sb * 2 * p + k + 2 * k * u for u in range(nruns)
                              if sb * 2 * p + k + 2 * k * u + k < L])
            out.append((p, k, streams))
            k //= 2
        p *= 2
    return out


def stream_pairs(st):
    if st[0] == 'blk':
        _, x0, k, bs, nb, run = st
        for b in range(nb):
            for r in range(run):
                yield (x0 + b * bs + r, x0 + b * bs + r + k)
    else:
        _, x0, k, bs2p, nsb, bs2k, nruns = st
        for sb in range(nsb):
            for u in range(nruns):
                for r in range(k):
                    yield (x0 + sb * bs2p + u * bs2k + r,
                           x0 + sb * bs2p + u * bs2k + r + k)


def validate_streams(L, n=256, e_flat=0):
    ref = oem_comparators(n)
    gen = gen_streams(L, n, e_flat=e_flat)
    for (refl, (p, k, sts)) in zip(ref, gen):
        want = sorted((a, b) for (a, b) in refl if b < L)
        got = sorted(pr for st in sts for pr in stream_pairs(st))
        assert got == want, ("oem stream gen mismatch", p, k)
    return gen


def build_plan(env_cnts, L, S, first_level_full=True, e_flat=None):
    """Item list per level. item = (kind, dims, lo_base, hi_base, K):
    kind 'tt' -> DVE min+max (both bases), 'cp' -> ScalarE copy lo->lo.
    dims = [(stride_cols, count), ...] outer->inner, <= 3 free dims."""
    env = np.sort(np.asarray(env_cnts))[::-1]
    assert len(env) == S

    def Keven(e):
        kk = int((env > e).sum())
        kk += kk % 2
        return min(S, kk)

    if e_flat is None:
        e_flat = int(env[env > 0].min()) if (env > 0).any() else 0
    levels = validate_streams(L, e_flat=e_flat)
    nlv_total = len(levels)
    plan = []
    for li, (p, k, sts) in enumerate(levels):
        # cap rect span in the last two levels so the interleaved output
        # DMA can fire progressively
        cap_cols = 32 * S if li >= nlv_total - 2 else None
        items = []
        touched = np.zeros(L, bool)
        for st in sts:
            for (a, b) in stream_pairs(st):
                touched[a] = touched[b] = True
            if st[0] == 'mrg':
                _, x0, kk, bs2p, nsb, bs2k, nruns = st
                dims = [(bs2p * S, nsb), (bs2k * S, nruns), (1, kk * S)]
                items.append(('tt', dims, x0 * S, (x0 + kk) * S, S))
                continue
            _, x0, kk, bs, nb, run = st
            if li == 0 and first_level_full:
                dims = [(bs * S, nb), (1, run * S)]
                items.append(('tt', dims, x0 * S, (x0 + kk) * S, S))
                continue
            if nb > 1:
                na, astride, pa = nb, bs, run
            else:
                na, astride, pa = run, 1, 1
            K1 = [Keven(x0 + a * astride + kk) for a in range(na)]
            K2 = [Keven(x0 + a * astride) for a in range(na)]
            INF = float('inf')
            best = [INF] * (na + 1)
            best[0] = 0.0
            choice = [None] * (na + 1)
            max_atoms = na
            if cap_cols is not None:
                max_atoms = max(1, cap_cols // max(1, astride * S))
            for a1 in range(1, na + 1):
                for a0 in range(a1 - 1, max(a1 - 1 - max_atoms, -1), -1):
                    K = K1[a0]
                    pairs = (a1 - a0) * pa
                    c = 0.0 if K == 0 else 2 * (DVE_CONST + DVE_ROW * pairs
                                                + DVE_EL * pairs * K)
                    if best[a0] + c < best[a1]:
                        best[a1] = best[a0] + c
                        choice[a1] = a0
            a1 = na
            rects = []
            while a1 > 0:
                a0 = choice[a1]
                rects.append((a0, a1))
                a1 = a0
            for (a0, a1) in reversed(rects):
                K = K1[a0]
                base = x0 + a0 * astride

                def mk(Kcols, koff):
                    dd = []
                    if a1 - a0 > 1:
                        dd.append((astride * S, a1 - a0))
                    if pa > 1:
                        dd.append((S, pa))
                    dd.append((1, Kcols))
                    return dd, (base + koff) * S
                if K > 0:
                    dims, b0c = mk(K, 0)
                    _, h0c = mk(K, kk)
                    items.append(('tt', dims, b0c, h0c, K))
                w2 = K2[a0]
                if w2 > K:
                    dims, b0c = mk(w2 - K, 0)
                    items.append(('cp', dims, b0c + K, None, w2 - K))
        # structural copies for positions untouched at this level
        un = np.nonzero(~touched)[0]
        segs = []
        for e in un:
            if segs and segs[-1][0] + segs[-1][1] == e:
                segs[-1][1] += 1
            else:
                segs.append([int(e), 1])
        fams = []
        for (st_, ln) in segs:
            if (fams and fams[-1][2] == ln
                    and fams[-1][3] != 0
                    and st_ - (fams[-1][0] + (fams[-1][1] - 1) * fams[-1][3])
                    == fams[-1][3]):
                fams[-1][1] += 1
            elif (fams and fams[-1][1] == 1 and fams[-1][2] == ln
                    and st_ - fams[-1][0] <= 48):
                fams[-1][3] = st_ - fams[-1][0]
                fams[-1][1] = 2
            else:
                fams.append([int(st_), 1, int(ln), 0])
        for (f0, nf, ln, gap) in fams:
            K = S if (li == 0 and first_level_full) else Keven(f0)
            if K == 0:
                continue
            if nf == 1:
                dims = [(S, ln), (1, K)] if K < S else [(1, ln * S)]
            else:
                dims = ([(gap * S, nf), (S, ln), (1, K)] if K < S
                        else [(gap * S, nf), (1, ln * S)])
            items.append(('cp', dims, f0 * S, None, K))
        # balance copy load: ScalarE runs ~1.25 cyc/elem @1.2GHz; when a
        # level's copy time would exceed ~70% of its DVE time, move the
        # largest copies to DVE as u32 tensor_copy (~0.31 cyc/elem @0.96).
        dve_ns = sum(2 * (DVE_CONST + DVE_EL * int(np.prod([c for _, c in d])))
                     for (kind, d, *_r) in [(i[0], i[1]) for i in items]
                     if kind == 'tt') / 0.96
        cps = [i for i in items if i[0] == 'cp']
        cps.sort(key=lambda i: -int(np.prod([c for _, c in i[1]])))
        act_ns = sum((260 + 1.25 * int(np.prod([c for _, c in i[1]]))) / 1.2
                     for i in cps)
        moved = set()
        for i in cps:
            if act_ns <= 1.3 * dve_ns:
                break
            fdv = int(np.prod([c for _, c in i[1]]))
            act_ns -= (260 + 1.25 * fdv) / 1.2
            moved.add(id(i))
        items = [(('cpd',) + i[1:]) if (i[0] == 'cp' and id(i) in moved)
                 else i for i in items]
        plan.append(items)
    return plan


def item_span(it):
    """(min_col, max_col) touched by an item, in column units."""
    kind, dims, lo, hi, K = it
    span = sum(st * (c - 1) for (st, c) in dims)
    if kind == 'tt':
        return (min(lo, hi), max(lo, hi) + span)
    return (lo, lo + span)


# ---------------------------------------------------------------------------
# Device kernel
# ---------------------------------------------------------------------------
_NC_CACHE = {}


def build_nc(env, L, S):
    key = (tuple(env), L, S)
    if key in _NC_CACHE:
        return _NC_CACHE[key]
    import concourse.bass as bass
    import concourse.bacc as bacc
    import concourse.mybir as mybir
    from concourse.tile import TileContext

    NCOL = S * L
    bf = mybir.dt.bfloat16
    plan = build_plan(np.asarray(env), L, S)

    nc = bacc.Bacc("TRN2", target_bir_lowering=False, debug=False,
                   num_devices=NCORES)
    xt = nc.declare_dram_parameter("xt", [512, NCOL], bf, isOutput=False)
    proj = nc.declare_dram_parameter("proj", [128, 64], bf, isOutput=False)
    out = nc.declare_dram_parameter("sorted", [256, NCOL], bf, isOutput=True)

    MM = 512          # matmul free chunk == one PSUM bank (fp32)
    EV = 2048         # eviction chunk (4 banks)
    CH = 3072 if NCOL <= 30000 else 2048

    with TileContext(nc) as tc:
        with (
            tc.tile_pool(name="const", bufs=1) as constp,
            tc.tile_pool(name="stage", bufs=2) as stagep,
            tc.tile_pool(name="psum", bufs=2, space="PSUM") as psump,
            tc.tile_pool(name="bufs", bufs=1) as bufp,
        ):
            projt = constp.tile([128, 64], bf)
            nc.sync.dma_start(projt[:], proj[:])

            bufA = bufp.tile([128, NCOL], bf, name="bufA", tag="bufA")
            bufB = bufp.tile([128, NCOL], bf, name="bufB", tag="bufB")
            bufZ = bufp.tile([128, NCOL], bf, name="bufZ", tag="bufZ")

            def fill(b, tgt, split_evict=False, ramp=False):
                """Generator: yields after each staged chunk so the caller
                can interleave emission with sort levels."""
                nev = 0
                c0 = 0
                ramp_sched = [1024, 1024, 2048] if ramp else []
                while c0 < NCOL:
                    cw = min(ramp_sched.pop(0) if ramp_sched else CH,
                             NCOL - c0)
                    sts = []
                    for ih in (0, 1):
                        i = 2 * b + ih
                        st = stagep.tile([128, CH], bf, name=f"st{ih}",
                                         tag=f"st{ih}")
                        nc.sync.dma_start(
                            st[:, :cw],
                            xt[i * 128:(i + 1) * 128, c0:c0 + cw])
                        sts.append(st)
                    e0 = 0
                    while e0 < cw:
                        ew = min(EV, cw - e0)
                        ps = psump.tile([128, EV], mybir.dt.float32,
                                        name="ps", tag="ps")
                        for ih in (0, 1):
                            j0 = 0
                            while j0 < ew:
                                jw = min(MM, ew - j0)
                                nc.tensor.matmul(
                                    ps[64 * ih:64 * ih + 64, j0:j0 + jw],
                                    lhsT=projt[:],
                                    rhs=sts[ih][:, e0 + j0:e0 + j0 + jw],
                                    start=True, stop=True)
                                j0 += jw
                        dst = tgt[:, c0 + e0:c0 + e0 + ew]
                        if split_evict and nev % 2 == 1:
                            nc.vector.tensor_copy(dst, ps[:, :ew])
                        else:
                            nc.scalar.copy(dst, ps[:, :ew])
                        nev += 1
                        e0 += ew
                    yield (c0, cw)
                    c0 += cw

            def mkap(buf_ap, col, dims):
                part = list(buf_ap.ap[0])
                return bass.AP(buf_ap.tensor, buf_ap.offset + col,
                               [part] + [[st, c] for (st, c) in dims])

            def emit_item(it, ca, pa):
                kind, dims, lo, hi, K = it
                if kind == 'tt':
                    slo = mkap(ca, lo, dims)
                    shi = mkap(ca, hi, dims)
                    nc.vector.tensor_tensor(mkap(pa, lo, dims), slo, shi,
                                            op=mybir.AluOpType.min)
                    nc.vector.tensor_tensor(mkap(pa, hi, dims), slo, shi,
                                            op=mybir.AluOpType.max)
                elif kind == 'cpd':
                    nc.vector.tensor_copy(
                        mkap(pa, lo, dims).bitcast(mybir.dt.uint32),
                        mkap(ca, lo, dims).bitcast(mybir.dt.uint32))
                else:
                    nc.scalar.copy(mkap(pa, lo, dims), mkap(ca, lo, dims))

            def emit_sort(cur, pong, out_row0, fill_gen=None, fill_start=3,
                          start_level=0):
                nlv = len(plan)
                for li in range(start_level, nlv - 2):
                    ca, pa = cur[:], pong[:]
                    for it in plan[li]:
                        emit_item(it, ca, pa)
                    if fill_gen is not None and li >= fill_start:
                        next(fill_gen, None)
                    cur, pong = pong, cur
                # last two levels interleaved in phases with progressive
                # output DMA. Level A (nlv-2): cur->pong; level B (nlv-1):
                # pong->cur; element e is final in `cur` once all level-B
                # items touching it are done.
                lA = sorted(plan[nlv - 2], key=lambda it: item_span(it)[0])
                lB = sorted(plan[nlv - 1], key=lambda it: item_span(it)[0])
                caA, paA = cur[:], pong[:]
                iA = iB = 0
                done_e = 0
                nph = 4
                for ph in range(nph):
                    last_ph = ph == nph - 1
                    b = (L * (ph + 1)) // nph
                    while iA < len(lA) and (last_ph or
                                            item_span(lA[iA])[0] // S < b):
                        emit_item(lA[iA], caA, paA)
                        iA += 1
                    while iB < len(lB) and (last_ph or
                                            item_span(lB[iB])[1] // S <= b - 2):
                        emit_item(lB[iB], paA, caA)
                        iB += 1
                    frontier = (item_span(lB[iB])[0] // S if iB < len(lB)
                                else L)
                    if frontier > done_e and (frontier - done_e >= 24
                                              or iB == len(lB)):
                        nc.sync.dma_start(
                            out[out_row0:out_row0 + 128,
                                done_e * S:frontier * S],
                            mkap(caA, done_e * S,
                                 [(1, (frontier - done_e) * S)]))
                        done_e = frontier
                assert done_e == L and iA == len(lA) and iB == len(lB)

            # fill A; interleave sort-A level 0 (pairs (2i,2i+1), full slot
            # width) chunk-by-chunk behind the PSUM evictions
            assert L % 2 == 0
            for (c0, cw) in fill(0, bufA, split_evict=True, ramp=True):
                ne = cw // S
                assert ne % 2 == 0 and cw % S == 0
                dims = [(2 * S, ne // 2), (1, S)]
                slo = mkap(bufA[:], c0, dims)
                shi = mkap(bufA[:], c0 + S, dims)
                nc.vector.tensor_tensor(mkap(bufZ[:], c0, dims), slo, shi,
                                        op=mybir.AluOpType.min)
                nc.vector.tensor_tensor(mkap(bufZ[:], c0 + S, dims), slo, shi,
                                        op=mybir.AluOpType.max)
            emit_sort(bufZ, bufA, 0, fill_gen=fill(1, bufB), start_level=1)
            emit_sort(bufB, bufZ, 128)

    nc.finalize()
    _NC_CACHE[key] = nc
    return nc


# ---------------------------------------------------------------------------
# Host side
# ---------------------------------------------------------------------------
def _plan_split(counts, spc):
    """Choose slots-per-core S (even) and slot length L: the largest
    segments are split across two slots (host merges their sorted halves),
    bounding L below the global max count. Minimizes S*L."""
    cs = np.sort(counts)[::-1]
    best = None
    for extra in range(0, 4):
        k = extra * NCORES
        Sv = spc + extra
        Sv += Sv % 2
        Lmin = int(np.ceil((cs[0] + 1) / 2)) if k else 0
        Lv = max(int(cs[k]) if k < len(cs) else 2, Lmin, 2)
        Lv += Lv % 2
        if Lv * 2 < cs[0] + 1 and k == 0:
            continue
        cost = Sv * Lv
        if best is None or cost < best[0]:
            best = (cost, Sv, Lv, k)
    _, Sv, Lv, k = best
    return Sv, Lv, k


def _host_prepare(x, batch, projections, cum_weights):
    N, DT = x.shape
    D, P = projections.shape
    I1 = DT // D
    Q = cum_weights.shape[0]
    counts = np.bincount(batch, minlength=G).astype(np.int64)
    starts = np.concatenate([[0], np.cumsum(counts)[:-1]]).astype(np.int64)
    spc = G // NCORES
    S, L, nsplit = _plan_split(counts, spc)

    qidx = np.floor(cum_weights[None, :].astype(np.float32)
                    * np.maximum(counts - 1, 0)[:, None].astype(np.float32)
                    ).astype(np.int64)
    scale = float((Q * P) ** (1.0 / POW))
    proj_s = np.ascontiguousarray(
        projections.astype(np.float32) / scale).astype(BF)
    proj_pad = np.zeros((128, 64), BF)
    proj_pad[:D, :P] = proj_s

    pf = projections.astype(np.float64)
    u_slice = pf @ np.linalg.solve(pf.T @ pf, np.full(P, BIG))
    u_row = np.tile(u_slice, I1).astype(np.float32)

    order = np.argsort(counts, kind="stable")
    split_set = set(order[G - nsplit:].tolist()) if nsplit else set()
    core_segs = [order[c::NCORES] for c in range(NCORES)]

    NCOL = S * L
    in_maps = []
    slot_tables = []
    core_cnts = []
    for c in range(NCORES):
        slots = []
        for g in core_segs[c]:
            cg = int(counts[g])
            if g in split_set:
                c1 = (cg + 1) // 2
                slots.append((g, 0, c1))
                slots.append((g, c1, cg - c1))
            else:
                slots.append((g, 0, cg))
        slots.sort(key=lambda t: -t[2])   # descending count
        while len(slots) < S:
            slots.append((-1, 0, 0))
        assert len(slots) == S
        slot_tables.append(slots)
        cnt_a = np.array([sl[2] for sl in slots])
        core_cnts.append(cnt_a)
        seg_a = np.array([sl[0] for sl in slots])
        off_a = np.array([sl[1] for sl in slots])
        st_a = np.where(seg_a >= 0, starts[np.clip(seg_a, 0, None)] + off_a, 0)
        e = np.arange(L)[:, None]
        v = e < cnt_a[None, :]                         # [L, S]
        ix = np.where(v, st_a[None, :] + e, 0)
        cols = np.where(v.reshape(-1, 1), x[ix.reshape(-1)], u_row[None, :])
        xtc = np.ascontiguousarray(cols.T.astype(BF))  # [512, NCOL]
        in_maps.append({"xt": xtc, "proj": proj_pad})
    env = np.max(np.stack(core_cnts), axis=0)
    return in_maps, dict(env=env, S=S, L=L, qidx=qidx, Q=Q,
                         P=P, I1=I1, slot_tables=slot_tables, NCOL=NCOL,
                         counts=counts)


def _host_gather(sorted_list, meta):
    Q, P, I1, L, S = meta["Q"], meta["P"], meta["I1"], meta["L"], meta["S"]
    qidx = meta["qidx"]
    counts = meta["counts"]
    out = np.empty((G, I1 * Q * P), np.float32)
    for c, srt in enumerate(sorted_list):
        a = np.asarray(srt).astype(np.float32)         # [256, S*L]
        blk = a.reshape(2, 2, 64, L, S).transpose(0, 1, 2, 4, 3)
        slots = meta["slot_tables"][c]
        one = [(si, sl[0]) for si, sl in enumerate(slots)
               if sl[0] >= 0 and sl[2] == counts[sl[0]]]
        if one:
            sidx = np.array([si for si, _ in one])
            segs = np.array([g for _, g in one])
            qs = qidx[segs]                            # [n, Q]
            sel = np.take_along_axis(blk[:, :, :, sidx, :],
                                     qs[None, None, None, :, :], axis=4)
            out[segs] = sel.transpose(3, 0, 1, 4, 2).reshape(len(segs),
                                                             I1 * Q * P)
        halves = {}
        for si, sl in enumerate(slots):
            if sl[0] >= 0 and sl[2] != counts[sl[0]]:
                halves.setdefault(sl[0], []).append((sl[1], si, sl[2]))
        for g, parts in halves.items():
            parts.sort()
            vals = np.concatenate([blk[:, :, :, si, :cnt]
                                   for _, si, cnt in parts], axis=3)
            vals = np.sort(vals, axis=3)               # [2,2,64,c_g]
            sel = vals[:, :, :, qidx[g]]               # [2,2,64,Q]
            out[g] = sel.transpose(0, 1, 3, 2).reshape(I1 * Q * P)
    return out


def _run_device(in_maps, meta, trace=False, tmpdir=None):
    from concourse.bass_utils import run_bass_kernel_spmd
    nc = build_nc(meta["env"], meta["L"], meta["S"])
    res = run_bass_kernel_spmd(nc, in_maps, core_ids=list(range(NCORES)),
                               trace=trace, tmpdir=tmpdir)
    return res


def kernel(x, batch, projections, cum_weights):
    x = np.asarray(x, dtype=np.float32)
    batch = np.asarray(batch)
    projections = np.asarray(projections, dtype=np.float32)
    cum_weights = np.asarray(cum_weights, dtype=np.float32)
    in_maps, meta = _host_prepare(x, batch, projections, cum_weights)
    res = _run_device(in_maps, meta)
    sorted_list = [res.results[c]["sorted"] for c in range(NCORES)]
    return _host_gather(sorted_list, meta)
